# revision 19
# baseline (speedup 1.0000x reference)
"""Trainium2 Bass kernel for nn_GumbelLayer: out = sigmoid((x@W.T + b + g1 - g2)/T).

g_i = -log(-log(u_i)), T = 0.1. Shapes: x,u1,u2,out [16384,1024]; W [1024,1024]; b [1024].
Data-parallel over 8 NeuronCores: each core handles 2048 batch rows; W/b replicated.

Device-side math per core (2048 rows = 16 tiles of 128 partitions):
  s      = ln(-ln(u2)) - ln(-ln(u1)) + b        (ACT x4 Ln passes + DVE sub/add)
  psum   = x @ W.T                              (PE, fp16 operands, fp32 accum)
  e      = psum + s                             (DVE)
  out    = sigmoid(10 * e) -> fp16              (ACT, scale fused)

Orchestration notes:
- ACT instruction order is forced to [all Ln][all Sigmoid] so walrus emits only
  two activation-table loads (Ln and Sigmoid live in different table sets).
- u1/u2/xt/out all ride the sync HWDGE ring in consumption-order FIFO; the
  weight chunks ride the gpsimd SWDGE ring so they can't head-of-line block u.
- Ln chunks are emitted two chunks ahead of the matmul tiles so the u stream
  stays ahead of the xt stream on the shared ring (ACT is the critical path).
- x/W are fp16 on the wire and in the matmul (error budget: ~4e-4 rms on the
  pre-sigmoid logit, ~2e-4 rms on the output; measured absmax ~4e-3).
"""
import sys

if '/opt/trn_rl_repo' not in sys.path:
    sys.path.insert(0, '/opt/trn_rl_repo')

import numpy as np

import concourse.bass as bass
import concourse.tile as tile
from concourse import bacc, mybir
from concourse.bass_utils import run_bass_kernel_spmd
from concourse.tile_rust import add_dep_helper

B, D = 16384, 1024
NCORES = 8
BS = B // NCORES          # 2048 rows per core
P = 128
BT = BS // P              # 16 row-tiles per core
KT = D // P               # 8 contraction chunks
N_HALF = 512              # matmul moving free-dim (one PSUM bank)
# progressive Ln chunk sizes (row-tiles): small first so ACT starts ~4us in,
# large later to amortize the 352-cycle ACTIVATE issue overhead
CHUNK_SIZES = (1, 1, 2, 4, 4, 4)
PIPE_OFFSET = 2           # ln chunks emitted this many chunks ahead of mm tiles
SIG_GROUP = 2             # row-tiles per sigmoid ACTIVATE
TEMP_INV = 10.0           # 1/T
OUT_FP16 = True           # store sigmoid output as fp16 (halves output DMA)

f32 = mybir.dt.float32
f16 = mybir.dt.float16
AF = mybir.ActivationFunctionType


def build_kernel():
    nc = bacc.Bacc("TRN2", target_bir_lowering=False, debug=False,
                   num_devices=NCORES)
    # xt[t, p, j*128+c] = x[t*128+c, j*128+p]  (pre-transposed on host, fp16)
    xt = nc.dram_tensor("xt", [BT, P, D], f16, kind="ExternalInput")
    u1 = nc.dram_tensor("u1", [BS, D], f32, kind="ExternalInput")
    u2 = nc.dram_tensor("u2", [BS, D], f32, kind="ExternalInput")
    wt = nc.dram_tensor("wt", [D, D], f16, kind="ExternalInput")   # W.T
    bbc = nc.dram_tensor("bbc", [P, D], f32, kind="ExternalInput")  # b row-bcast
    out = nc.dram_tensor("out", [BS, D], f16 if OUT_FP16 else f32,
                         kind="ExternalOutput")

    with tile.TileContext(nc) as tc:
        _body(tc, nc, xt, u1, u2, wt, bbc, out)
    nc.compile()
    return nc


def _body(tc, nc, xt, u1, u2, wt, bbc, out):
    with (
        tc.tile_pool(name="const", bufs=1) as cpool,
        tc.tile_pool(name="wts", bufs=1) as wpool,
        tc.tile_pool(name="sslab", bufs=1) as spool,
        tc.tile_pool(name="uin", bufs=2) as upool,
        tc.tile_pool(name="lntmp", bufs=2) as lpool,
        tc.tile_pool(name="xin", bufs=4) as xpool,
        tc.tile_pool(name="oout", bufs=4) as opool,
        tc.tile_pool(name="ps", bufs=4, space="PSUM") as pspool,
    ):
        bbt = cpool.tile([P, D], f32)
        nc.gpsimd.dma_start(bbt[:], bbc.ap()[:])

        # W.T resident in SBUF: wts[p, j, o] = W.T[j*128+p, o], fp16
        wts = wpool.tile([P, KT, D], f16)
        wtr = wt.ap().rearrange("(j p) o -> p j o", p=P)
        for j in range(KT):
            nc.gpsimd.dma_start(wts[:, j, :], wtr[:, j, :])

        # persistent slab: s[p, t, o] = gumbel diff + bias, all 16 row-tiles
        s_slab = spool.tile([P, BT, D], f32)

        u1r = u1.ap().rearrange("(n p) d -> p n d", p=P)   # [128, 16, 1024]
        u2r = u2.ap().rearrange("(n p) d -> p n d", p=P)
        outr = out.ap().rearrange("(n p) d -> p n d", p=P)

        ln_insts = []
        ch_max = max(CHUNK_SIZES)

        def emit_ln_chunk(t0, ch):
            sl = slice(t0, t0 + ch)
            # d1 = ln(-ln(u1)) -> s_slab
            uc1 = upool.tile([P, ch_max, D], f32, tag="u")
            nc.sync.dma_start(uc1[:, :ch, :], u1r[:, sl, :])
            lt1 = lpool.tile([P, ch_max, D], f32, tag="ln")
            nc.scalar.activation(lt1[:, :ch, :], uc1[:, :ch, :], AF.Ln)
            ln_insts.append(
                nc.scalar.activation(s_slab[:, sl, :], lt1[:, :ch, :], AF.Ln,
                                     scale=-1.0))
            # d2 = ln(-ln(u2)); s = d2 - d1; s += b
            uc2 = upool.tile([P, ch_max, D], f32, tag="u")
            nc.sync.dma_start(uc2[:, :ch, :], u2r[:, sl, :])
            lt2 = lpool.tile([P, ch_max, D], f32, tag="ln")
            nc.scalar.activation(lt2[:, :ch, :], uc2[:, :ch, :], AF.Ln)
            ln_insts.append(
                nc.scalar.activation(lt2[:, :ch, :], lt2[:, :ch, :], AF.Ln,
                                     scale=-1.0))
            nc.vector.tensor_sub(s_slab[:, sl, :], lt2[:, :ch, :],
                                 s_slab[:, sl, :])
            for t in range(t0, t0 + ch):
                nc.vector.tensor_add(s_slab[:, t, :], s_slab[:, t, :], bbt[:])

        def emit_mm_tile(t):
            xts = xpool.tile([P, D], f16)
            nc.sync.dma_start(xts[:], xt.ap()[t])
            psum = pspool.tile([P, D], f32)
            for j in range(KT):
                for n in range(2):
                    nsl = slice(n * N_HALF, (n + 1) * N_HALF)
                    nc.tensor.matmul(
                        psum[:, nsl],
                        xts[:, j * P:(j + 1) * P],
                        wts[:, j, nsl],
                        start=(j == 0), stop=(j == KT - 1))
            nc.vector.tensor_add(s_slab[:, t, :], psum[:], s_slab[:, t, :])

        # pipeline: ln chunks run PIPE_OFFSET chunks ahead of matmul tiles
        chunk_starts = []
        t0 = 0
        for ch in CHUNK_SIZES:
            chunk_starts.append((t0, ch))
            t0 += ch
        n_ch = len(CHUNK_SIZES)
        for ci in range(n_ch + PIPE_OFFSET):
            if ci < n_ch:
                emit_ln_chunk(*chunk_starts[ci])
            if ci >= PIPE_OFFSET:
                mm_t0, mm_ch = chunk_starts[ci - PIPE_OFFSET]
                for t in range(mm_t0, mm_t0 + mm_ch):
                    emit_mm_tile(t)

        # ---- sigmoid + store (ACT table set switches once, after all Ln) ----
        last_ln = ln_insts[-1]
        for t in range(0, BT, SIG_GROUP):
            ot = opool.tile([P, SIG_GROUP, D], f16 if OUT_FP16 else f32)
            sig = nc.scalar.activation(ot[:], s_slab[:, t:t + SIG_GROUP, :],
                                       AF.Sigmoid, scale=TEMP_INV)
            add_dep_helper(sig.ins, last_ln.ins, sync=False,
                           reason="ACT table-set phase ordering")
            nc.sync.dma_start(outr[:, t:t + SIG_GROUP, :], ot[:])


_NC_CACHE = None


def _get_nc():
    global _NC_CACHE
    if _NC_CACHE is None:
        _NC_CACHE = build_kernel()
    return _NC_CACHE


def _prep_core_inputs(x_c, u1_c, u2_c, wt_np, bbc_np):
    # xt[t, p, j*128+c] = x[t*128+c, j*128+p]
    xt_c = np.ascontiguousarray(
        x_c.reshape(BT, P, KT, P).transpose(0, 3, 2, 1).reshape(BT, P, D)
        .astype(np.float16))
    return {"xt": xt_c, "u1": np.ascontiguousarray(u1_c),
            "u2": np.ascontiguousarray(u2_c), "wt": wt_np, "bbc": bbc_np}


def run(x, u1, u2, W, b, trace=False, **trace_kwargs):
    nc = _get_nc()
    x = np.asarray(x, dtype=np.float32)
    u1 = np.asarray(u1, dtype=np.float32)
    u2 = np.asarray(u2, dtype=np.float32)
    wt_np = np.ascontiguousarray(
        np.asarray(W, dtype=np.float32).T.astype(np.float16))
    bbc_np = np.ascontiguousarray(np.broadcast_to(
        np.asarray(b, dtype=np.float32).reshape(1, D), (P, D)))
    in_maps = []
    for c in range(NCORES):
        sl = slice(c * BS, (c + 1) * BS)
        in_maps.append(
            _prep_core_inputs(x[sl], u1[sl], u2[sl], wt_np, bbc_np))
    res = run_bass_kernel_spmd(nc, in_maps, list(range(NCORES)),
                               trace=trace, **trace_kwargs)
    out = np.concatenate([res.results[c]["out"] for c in range(NCORES)], axis=0)
    return out.astype(np.float32), res


def kernel(x, u1, u2, W, b, with_grad=None):
    out, _ = run(x, u1, u2, W, b)
    return out


# revision 20
# speedup vs baseline: 1.0737x; 1.0737x over previous
"""Trainium2 Bass kernel for nn_GumbelLayer: out = sigmoid((x@W.T + b + g1 - g2)/T).

g_i = -log(-log(u_i)), T = 0.1. Shapes: x,u1,u2,out [16384,1024]; W [1024,1024]; b [1024].
Data-parallel over 8 NeuronCores: each core handles 2048 batch rows; W/b replicated.

Device-side math per core (2048 rows = 16 tiles of 128 partitions):
  s      = ln(-ln(u2)) - ln(-ln(u1)) + b        (ACT x4 Ln passes + DVE sub/add)
  psum   = x @ W.T                              (PE, fp16 operands, fp32 accum)
  e      = psum + s                             (DVE)
  out    = sigmoid(10 * e) -> fp16              (ACT, scale fused)

Orchestration notes:
- ACT instruction order is forced to [all Ln][all Sigmoid] so walrus emits only
  two activation-table loads (Ln and Sigmoid live in different table sets).
- u1/u2/xt/out all ride the sync HWDGE ring in consumption-order FIFO; the
  weight chunks ride the gpsimd SWDGE ring so they can't head-of-line block u.
- Ln chunks are emitted two chunks ahead of the matmul tiles so the u stream
  stays ahead of the xt stream on the shared ring (ACT is the critical path).
- x/W are fp16 on the wire and in the matmul (error budget: ~4e-4 rms on the
  pre-sigmoid logit, ~2e-4 rms on the output; measured absmax ~4e-3).
"""
import sys

if '/opt/trn_rl_repo' not in sys.path:
    sys.path.insert(0, '/opt/trn_rl_repo')

import numpy as np

import concourse.bass as bass
import concourse.tile as tile
from concourse import bacc, mybir
from concourse.bass_utils import run_bass_kernel_spmd
from concourse.tile_rust import add_dep_helper

B, D = 16384, 1024
NCORES = 8
BS = B // NCORES          # 2048 rows per core
P = 128
BT = BS // P              # 16 row-tiles per core
KT = D // P               # 8 contraction chunks
N_HALF = 512              # matmul moving free-dim (one PSUM bank)
# progressive Ln chunk sizes (row-tiles): small first so ACT starts ~4us in,
# large later to amortize the 352-cycle ACTIVATE issue overhead
CHUNK_SIZES = (1, 1, 2, 4, 4, 4)
PIPE_OFFSET = 0           # ln chunks emitted this many chunks ahead of mm tiles
SIG_GROUP = 2             # row-tiles per sigmoid ACTIVATE
TEMP_INV = 10.0           # 1/T
OUT_FP16 = True           # store sigmoid output as fp16 (halves output DMA)

f32 = mybir.dt.float32
f16 = mybir.dt.float16
AF = mybir.ActivationFunctionType


def build_kernel():
    nc = bacc.Bacc("TRN2", target_bir_lowering=False, debug=False,
                   num_devices=NCORES)
    # xt[t, p, j*128+c] = x[t*128+c, j*128+p]  (pre-transposed on host, fp16)
    xt = nc.dram_tensor("xt", [BT, P, D], f16, kind="ExternalInput")
    u1 = nc.dram_tensor("u1", [BS, D], f32, kind="ExternalInput")
    u2 = nc.dram_tensor("u2", [BS, D], f32, kind="ExternalInput")
    wt = nc.dram_tensor("wt", [D, D], f16, kind="ExternalInput")   # W.T
    bbc = nc.dram_tensor("bbc", [P, D], f32, kind="ExternalInput")  # b row-bcast
    out = nc.dram_tensor("out", [BS, D], f16 if OUT_FP16 else f32,
                         kind="ExternalOutput")

    with tile.TileContext(nc) as tc:
        _body(tc, nc, xt, u1, u2, wt, bbc, out)
    nc.compile()
    return nc


def _body(tc, nc, xt, u1, u2, wt, bbc, out):
    with (
        tc.tile_pool(name="const", bufs=1) as cpool,
        tc.tile_pool(name="wts", bufs=1) as wpool,
        tc.tile_pool(name="sslab", bufs=1) as spool,
        tc.tile_pool(name="uin", bufs=2) as upool,
        tc.tile_pool(name="lntmp", bufs=2) as lpool,
        tc.tile_pool(name="xin", bufs=4) as xpool,
        tc.tile_pool(name="oout", bufs=4) as opool,
        tc.tile_pool(name="ps", bufs=4, space="PSUM") as pspool,
    ):
        bbt = cpool.tile([P, D], f32)
        nc.gpsimd.dma_start(bbt[:], bbc.ap()[:])

        # W.T resident in SBUF: wts[p, j, o] = W.T[j*128+p, o], fp16
        wts = wpool.tile([P, KT, D], f16)
        wtr = wt.ap().rearrange("(j p) o -> p j o", p=P)
        for j in range(KT):
            nc.gpsimd.dma_start(wts[:, j, :], wtr[:, j, :])

        # persistent slab: s[p, t, o] = gumbel diff + bias, all 16 row-tiles
        s_slab = spool.tile([P, BT, D], f32)

        u1r = u1.ap().rearrange("(n p) d -> p n d", p=P)   # [128, 16, 1024]
        u2r = u2.ap().rearrange("(n p) d -> p n d", p=P)
        outr = out.ap().rearrange("(n p) d -> p n d", p=P)

        ln_insts = []
        ch_max = max(CHUNK_SIZES)

        def emit_ln_chunk(t0, ch):
            sl = slice(t0, t0 + ch)
            # d1 = ln(-ln(u1)) -> s_slab
            uc1 = upool.tile([P, ch_max, D], f32, tag="u")
            nc.sync.dma_start(uc1[:, :ch, :], u1r[:, sl, :])
            lt1 = lpool.tile([P, ch_max, D], f32, tag="ln")
            nc.scalar.activation(lt1[:, :ch, :], uc1[:, :ch, :], AF.Ln)
            ln_insts.append(
                nc.scalar.activation(s_slab[:, sl, :], lt1[:, :ch, :], AF.Ln,
                                     scale=-1.0))
            # d2 = ln(-ln(u2)); s = d2 - d1; s += b
            uc2 = upool.tile([P, ch_max, D], f32, tag="u")
            nc.sync.dma_start(uc2[:, :ch, :], u2r[:, sl, :])
            lt2 = lpool.tile([P, ch_max, D], f32, tag="ln")
            nc.scalar.activation(lt2[:, :ch, :], uc2[:, :ch, :], AF.Ln)
            ln_insts.append(
                nc.scalar.activation(lt2[:, :ch, :], lt2[:, :ch, :], AF.Ln,
                                     scale=-1.0))
            nc.vector.tensor_sub(s_slab[:, sl, :], lt2[:, :ch, :],
                                 s_slab[:, sl, :])
            for t in range(t0, t0 + ch):
                nc.vector.tensor_add(s_slab[:, t, :], s_slab[:, t, :], bbt[:])

        def emit_mm_tile(t):
            xts = xpool.tile([P, D], f16)
            nc.gpsimd.dma_start(xts[:], xt.ap()[t])
            psum = pspool.tile([P, D], f32)
            for j in range(KT):
                for n in range(2):
                    nsl = slice(n * N_HALF, (n + 1) * N_HALF)
                    nc.tensor.matmul(
                        psum[:, nsl],
                        xts[:, j * P:(j + 1) * P],
                        wts[:, j, nsl],
                        start=(j == 0), stop=(j == KT - 1))
            nc.vector.tensor_add(s_slab[:, t, :], psum[:], s_slab[:, t, :])

        # pipeline: ln chunks run PIPE_OFFSET chunks ahead of matmul tiles
        chunk_starts = []
        t0 = 0
        for ch in CHUNK_SIZES:
            chunk_starts.append((t0, ch))
            t0 += ch
        n_ch = len(CHUNK_SIZES)
        for ci in range(n_ch + PIPE_OFFSET):
            if ci < n_ch:
                emit_ln_chunk(*chunk_starts[ci])
            if ci >= PIPE_OFFSET:
                mm_t0, mm_ch = chunk_starts[ci - PIPE_OFFSET]
                for t in range(mm_t0, mm_t0 + mm_ch):
                    emit_mm_tile(t)

        # ---- sigmoid + store (ACT table set switches once, after all Ln) ----
        last_ln = ln_insts[-1]
        for t in range(0, BT, SIG_GROUP):
            ot = opool.tile([P, SIG_GROUP, D], f16 if OUT_FP16 else f32)
            sig = nc.scalar.activation(ot[:], s_slab[:, t:t + SIG_GROUP, :],
                                       AF.Sigmoid, scale=TEMP_INV)
            add_dep_helper(sig.ins, last_ln.ins, sync=False,
                           reason="ACT table-set phase ordering")
            nc.sync.dma_start(outr[:, t:t + SIG_GROUP, :], ot[:])


_NC_CACHE = None


def _get_nc():
    global _NC_CACHE
    if _NC_CACHE is None:
        _NC_CACHE = build_kernel()
    return _NC_CACHE


def _prep_core_inputs(x_c, u1_c, u2_c, wt_np, bbc_np):
    # xt[t, p, j*128+c] = x[t*128+c, j*128+p]
    xt_c = np.ascontiguousarray(
        x_c.reshape(BT, P, KT, P).transpose(0, 3, 2, 1).reshape(BT, P, D)
        .astype(np.float16))
    return {"xt": xt_c, "u1": np.ascontiguousarray(u1_c),
            "u2": np.ascontiguousarray(u2_c), "wt": wt_np, "bbc": bbc_np}


def run(x, u1, u2, W, b, trace=False, **trace_kwargs):
    nc = _get_nc()
    x = np.asarray(x, dtype=np.float32)
    u1 = np.asarray(u1, dtype=np.float32)
    u2 = np.asarray(u2, dtype=np.float32)
    wt_np = np.ascontiguousarray(
        np.asarray(W, dtype=np.float32).T.astype(np.float16))
    bbc_np = np.ascontiguousarray(np.broadcast_to(
        np.asarray(b, dtype=np.float32).reshape(1, D), (P, D)))
    in_maps = []
    for c in range(NCORES):
        sl = slice(c * BS, (c + 1) * BS)
        in_maps.append(
            _prep_core_inputs(x[sl], u1[sl], u2[sl], wt_np, bbc_np))
    res = run_bass_kernel_spmd(nc, in_maps, list(range(NCORES)),
                               trace=trace, **trace_kwargs)
    out = np.concatenate([res.results[c]["out"] for c in range(NCORES)], axis=0)
    return out.astype(np.float32), res


def kernel(x, u1, u2, W, b, with_grad=None):
    out, _ = run(x, u1, u2, W, b)
    return out
